# revision 21
# baseline (speedup 1.0000x reference)
"""MoE Top-K router kernel for Trainium2 (8 NeuronCores, data-parallel).

reference:
    logits  = X @ W.T                    # [T,H]@[H,E] -> [T,E] fp32
    weights = softmax(logits, axis=-1)   # fp32
    indices = top_k(weights, 8).indices  # int32, sorted by weight desc

Sharding: tokens split 8 ways (2048 tokens/core); W replicated.

Matmul runs in fp16 hi/lo split form for full fp32-grade accuracy at fp16 PE
throughput (fp32 matmul is 4 cycles/row and its 4-byte weight loads don't get
FWL; fp16 gets 1 cycle/row + fast weight load):
    X = Xhi + Xlo (fp16 pair), W' = W*64 = Whi + Wlo (fp16 pair; the *64
    power-of-two prescale keeps Wlo out of fp16 subnormals and is undone
    exactly by scale=1/64 on the ACT ops).
    X @ W'.T ~= Xhi@Whi + Xhi@Wlo + Xlo@Whi   (Xlo@Wlo ~ 2^-22, dropped)
Per chunk the hi weight-load is shared: one N=128 matmul with rhs=[Whi|Wlo]
(cols 0:64 / 64:128 of PSUM) plus one N=64 matmul for Xlo@Whi; the two column
halves are folded afterwards. Measured absmax error vs fp64 on the real data:
2.1e-6 (fp32 direct matmul: 2.4e-6).

Softmax skips the max-subtraction: logits here are bounded (|x| < ~20 even at
5+ sigma), exp stays far inside fp32 range, and softmax is shift-invariant, so
exp(x)/sum(exp(x)) rounds within ~1 ulp of the reference's exp(x-m) path.
That lets the whole epilogue run batched over 4 token-tiles sharing one PSUM
bank: fold-copy, add, logits copy (scale=1/64), exp, row-sum, reciprocal and
softmax scale are each ONE instruction per 4 tiles; only top-8 (InstMax /
InstMaxIndex, which flatten their free dims) runs per tile. InstMax/
InstMaxIndex match jax.lax.top_k tie semantics exactly (values descending,
ties by ascending index).

Host-side prep transposes each token-shard to contraction-major layout so the
device does zero transposes:
    xhl[g, p, c*128 + t]        = Xhi_shard[g*128 + t, c*128 + p]
    xhl[g, p, H + c*128 + t]    = Xlo_shard[g*128 + t, c*128 + p]
    whl[p, c*128 + e]           = Whi[e, c*128 + p]   (e < 64)
    whl[p, c*128 + 64 + e]      = Wlo[e, c*128 + p]
Each tile is one 2 MiB DMA (16 KiB contiguous per partition), alternating
between the two HWDGE rings (SP/ACT); outputs stream back incrementally on
the ACT ring, overlapped with the input stream.
Measured: ~117-125 us HW exec per core; the input stream (32 MiB/core over
~358 GB/s HBM-per-core) runs at ~96% DMA busy, which is the roofline for this
shape. Startup preamble (~8 us) and the Tile drain/barrier tail (~10 us) are
framework-fixed costs.
"""

import numpy as np

import concourse.bass as bass
import concourse.bacc as bacc
import concourse.mybir as mybir
from concourse.tile import TileContext
from concourse.bass_utils import run_bass_kernel_spmd

T, H, E, TOPK = 16384, 4096, 64, 8
NCORES = 8
TC = T // NCORES          # 2048 tokens per core
PT = 128                  # tokens per tile (partition dim)
NG = TC // PT             # 16 tiles per core
NCH = H // 128            # 32 contraction chunks
BATCH_SCHEDULE = (4, 4, 4, 2, 1, 1)  # tiles per PSUM-bank batch; small tail
WSCALE = 64.0             # power-of-two W prescale (exactly undone on device)

F32 = mybir.dt.float32
F16 = mybir.dt.float16
U32 = mybir.dt.uint32


def build(x_bufs: int = 8, psum_bufs: int = 2, alt_rings: bool = True):
    nc = bacc.Bacc()
    xhl = nc.dram_tensor("xhl", [NG, 128, 2 * H], F16, kind="ExternalInput")
    whl = nc.dram_tensor("whl", [128, NCH * 128], F16, kind="ExternalInput")
    logits = nc.dram_tensor("logits", [TC, E], F32, kind="ExternalOutput")
    weights = nc.dram_tensor("weights", [TC, E], F32, kind="ExternalOutput")
    indices = nc.dram_tensor("indices", [TC, TOPK], U32, kind="ExternalOutput")

    inv = 1.0 / WSCALE
    lg_view = logits.rearrange("(g p) e -> p g e", p=128)
    wg_view = weights.rearrange("(g p) e -> p g e", p=128)
    ix_view = indices.rearrange("(g p) k -> p g k", p=128)

    with TileContext(nc) as tc:
        with (
            tc.tile_pool(name="xp", bufs=x_bufs) as xp,
            tc.tile_pool(name="wp", bufs=1) as wp,
            tc.tile_pool(name="pp", bufs=psum_bufs, space="PSUM") as pp,
            tc.tile_pool(name="res", bufs=1) as res,
            tc.tile_pool(name="tmp", bufs=2) as tmp,
            tc.tile_pool(name="st", bufs=3) as st,
        ):
            # weights on the ACT HWDGE ring so the SP ring starts streaming X
            # immediately
            wt_sb = wp.tile([128, NCH * 128], F16)
            nc.scalar.dma_start(out=wt_sb[:], in_=whl[:, :])

            lg_all = res.tile([128, NG * E], F32, tag="lg")
            wg_all = res.tile([128, NG * E], F32, tag="wg")
            idx_all = res.tile([128, NG * TOPK], U32, tag="idx")

            # full-width batches in the body; shrinking batches at the end so
            # the final epilogue chain (which serializes after the last
            # matmul) is as short as possible
            batches = []
            g0 = 0
            for tb_n in BATCH_SCHEDULE:
                batches.append((g0, tb_n))
                g0 += tb_n
            assert g0 == NG

            for g0, tb_n in batches:
                xgs = []
                for tb in range(tb_n):
                    g = g0 + tb
                    xg = xp.tile([128, 2 * H], F16, tag="xg")
                    # alternate the two HWDGE rings so descriptor-gen and
                    # completion handling pipeline across rings
                    eng = nc.sync if (g % 2 == 0 or not alt_rings) else nc.scalar
                    eng.dma_start(out=xg[:], in_=xhl[g])
                    xgs.append(xg)

                # one PSUM bank holds tb_n tiles x [Whi-cols | Wlo-cols]
                ps = pp.tile([128, tb_n * 2 * E], F32, tag="ps")
                for tb in range(tb_n):
                    xg = xgs[tb]
                    o = tb * 2 * E
                    for c in range(NCH):
                        hi = xg[:, c * 128:(c + 1) * 128]
                        lo = xg[:, H + c * 128:H + (c + 1) * 128]
                        whilo = wt_sb[:, c * 128:(c + 1) * 128]
                        whi = wt_sb[:, c * 128:c * 128 + 64]
                        nc.tensor.matmul(
                            ps[:, o:o + 2 * E], lhsT=hi, rhs=whilo,
                            start=(c == 0), stop=False, skip_group_check=True,
                        )
                        nc.tensor.matmul(
                            ps[:, o:o + E], lhsT=lo, rhs=whi,
                            start=False, stop=(c == NCH - 1),
                            skip_group_check=True,
                        )

                # ---- batched epilogue over tb_n tiles ----
                ps3 = ps[:].rearrange("p (t u) -> p t u", u=2 * E)
                half = tmp.tile([128, tb_n * E], F32, tag="half")
                half3 = half[:].rearrange("p (t u) -> p t u", u=E)
                nc.scalar.activation(
                    out=half3, in_=ps3[:, :, E:2 * E],
                    func=mybir.ActivationFunctionType.Copy,
                )
                lgp = tmp.tile([128, tb_n * E], F32, tag="lgp")
                lgp3 = lgp[:].rearrange("p (t u) -> p t u", u=E)
                nc.vector.tensor_add(lgp3, ps3[:, :, 0:E], half3)

                # exp first: the logits copy below is not needed by the
                # softmax/top-k chain, keep it off the critical path
                ex = tmp.tile([128, tb_n * E], F32, tag="ex")
                nc.scalar.activation(
                    out=ex[:], in_=lgp[:],
                    func=mybir.ActivationFunctionType.Exp, scale=inv,
                )
                ex3 = ex[:].rearrange("p (t u) -> p t u", u=E)

                s4 = st.tile([128, tb_n], F32, tag="s4")
                nc.vector.tensor_reduce(
                    out=s4[:], in_=ex3, axis=mybir.AxisListType.X,
                    op=mybir.AluOpType.add,
                )
                r4 = st.tile([128, tb_n], F32, tag="r4")
                nc.vector.reciprocal(r4[:], s4[:])

                wgf = wg_all[:, g0 * E:(g0 + tb_n) * E]
                wg3 = wgf.rearrange("p (t u) -> p t u", u=E)
                nc.vector.tensor_mul(wg3, ex3, r4[:].to_broadcast([128, tb_n, E]))

                for tb in range(tb_n):
                    g = g0 + tb
                    wg = wg_all[:, g * E:(g + 1) * E]
                    top8 = st.tile([128, TOPK], F32, tag="top8")
                    nc.vector.max(out=top8[:], in_=wg)
                    idx = idx_all[:, g * TOPK:(g + 1) * TOPK]
                    nc.vector.max_index(out=idx, in_max=top8[:], in_values=wg)

                lg = lg_all[:, g0 * E:(g0 + tb_n) * E]
                nc.scalar.activation(
                    out=lg, in_=lgp[:],
                    func=mybir.ActivationFunctionType.Copy, scale=inv,
                )

                # incremental writeback on the ACT ring, overlapped with the
                # input stream; indices first (end of the critical chain)
                gs = slice(g0, g0 + tb_n)
                fs = slice(g0 * E, (g0 + tb_n) * E)
                ks = slice(g0 * TOPK, (g0 + tb_n) * TOPK)
                nc.scalar.dma_start(out=ix_view[:, gs, :], in_=idx_all[:, ks])
                nc.scalar.dma_start(out=wg_view[:, gs, :], in_=wg_all[:, fs])
                nc.scalar.dma_start(out=lg_view[:, gs, :], in_=lg_all[:, fs])

    nc.finalize()
    return nc


_NC_CACHE = None
LAST_EXEC_NS = None


def _get_nc():
    global _NC_CACHE
    if _NC_CACHE is None:
        _NC_CACHE = build()
    return _NC_CACHE


def _prep_w(W: np.ndarray):
    # W prescale + fp16 hi/lo split, contraction-major: whl[p, c, 0:64|64:128]
    Wp = (W * WSCALE).astype(np.float32)
    W1 = np.ascontiguousarray(Wp.reshape(E, NCH, 128).transpose(2, 1, 0))  # [128,NCH,64]
    whi = W1.astype(np.float16)
    wlo = (W1 - whi.astype(np.float32)).astype(np.float16)
    return np.concatenate([whi, wlo], axis=2).reshape(128, NCH * 128)


def _prep_x_numpy(hidden_states: np.ndarray):
    xhi_full = hidden_states.astype(np.float16)
    xlo_full = (hidden_states - xhi_full.astype(np.float32)).astype(np.float16)
    out = []
    for core in range(NCORES):
        xhl = np.empty((NG, 128, 2 * H), np.float16)
        dst_hi = xhl[:, :, :H].reshape(NG, 128, NCH, PT)
        dst_lo = xhl[:, :, H:].reshape(NG, 128, NCH, PT)
        sl = slice(core * TC, (core + 1) * TC)
        # [g, t, c, p] -> [g, p, c, t]
        dst_hi[...] = xhi_full[sl].reshape(NG, PT, NCH, 128).transpose(0, 3, 2, 1)
        dst_lo[...] = xlo_full[sl].reshape(NG, PT, NCH, 128).transpose(0, 3, 2, 1)
        out.append(xhl)
    return out


def _prep_x(hidden_states: np.ndarray):
    # XLA-CPU does the big permute blocked + multithreaded (~4x numpy)
    try:
        import jax
        import jax.numpy as jnp

        cpu = jax.devices("cpu")[0]

        @jax.jit
        def prep_all(x):
            xt = x.reshape(NCORES, NG, PT, NCH, 128).transpose(0, 1, 4, 3, 2)
            xt = xt.reshape(NCORES, NG, 128, H)
            hi = xt.astype(jnp.float16)
            lo = (xt - hi.astype(jnp.float32)).astype(jnp.float16)
            return jnp.concatenate([hi, lo], axis=3)

        with jax.default_device(cpu):
            out = np.asarray(prep_all(hidden_states))
        return [out[core] for core in range(NCORES)]
    except Exception:
        return _prep_x_numpy(hidden_states)


def _prep_core_inputs(hidden_states: np.ndarray, W: np.ndarray):
    whl = _prep_w(W)
    xs = _prep_x(hidden_states)
    return [{"xhl": xs[core], "whl": whl} for core in range(NCORES)]


def kernel(hidden_states: np.ndarray, W: np.ndarray):
    hidden_states = np.ascontiguousarray(hidden_states, dtype=np.float32)
    W = np.ascontiguousarray(W, dtype=np.float32)
    assert hidden_states.shape == (T, H) and W.shape == (E, H)

    nc = _get_nc()
    in_maps = _prep_core_inputs(hidden_states, W)
    res = run_bass_kernel_spmd(nc, in_maps, core_ids=list(range(NCORES)))
    global LAST_EXEC_NS
    if res.exec_time_ns is not None:
        LAST_EXEC_NS = res.exec_time_ns

    logits = np.concatenate([res.results[i]["logits"] for i in range(NCORES)], axis=0)
    weights = np.concatenate([res.results[i]["weights"] for i in range(NCORES)], axis=0)
    indices = np.concatenate(
        [res.results[i]["indices"] for i in range(NCORES)], axis=0
    ).astype(np.int32)
    return logits, weights, indices


# revision 24
# speedup vs baseline: 1.0294x; 1.0294x over previous
"""MoE Top-K router kernel for Trainium2 (8 NeuronCores, data-parallel).

reference:
    logits  = X @ W.T                    # [T,H]@[H,E] -> [T,E] fp32
    weights = softmax(logits, axis=-1)   # fp32
    indices = top_k(weights, 8).indices  # int32, sorted by weight desc

Sharding: tokens split 8 ways (2048 tokens/core); W replicated.

Matmul runs in fp16 hi/lo split form for full fp32-grade accuracy at fp16 PE
throughput (fp32 matmul is 4 cycles/row and its 4-byte weight loads don't get
FWL; fp16 gets 1 cycle/row + fast weight load):
    X = Xhi + Xlo (fp16 pair), W' = W*64 = Whi + Wlo (fp16 pair; the *64
    power-of-two prescale keeps Wlo out of fp16 subnormals and is undone
    exactly by scale=1/64 on the ACT ops).
    X @ W'.T ~= Xhi@Whi + Xhi@Wlo + Xlo@Whi   (Xlo@Wlo ~ 2^-22, dropped)
Per chunk the hi weight-load is shared: one N=128 matmul with rhs=[Whi|Wlo]
(cols 0:64 / 64:128 of PSUM) plus one N=64 matmul for Xlo@Whi; the two column
halves are folded afterwards. Measured absmax error vs fp64 on the real data:
2.1e-6 (fp32 direct matmul: 2.4e-6).

Softmax skips the max-subtraction: logits here are bounded (|x| < ~20 even at
5+ sigma), exp stays far inside fp32 range, and softmax is shift-invariant, so
exp(x)/sum(exp(x)) rounds within ~1 ulp of the reference's exp(x-m) path.
That lets the whole epilogue run batched over up to 4 token-tiles sharing one
PSUM bank: fold-copy, add, logits copy (scale=1/64), exp, row-sum, reciprocal and
softmax scale are each ONE instruction per 4 tiles; only top-8 (InstMax /
InstMaxIndex, which flatten their free dims) runs per tile. InstMax/
InstMaxIndex match jax.lax.top_k tie semantics exactly (values descending,
ties by ascending index).

Host-side prep transposes each token-shard to contraction-major layout so the
device does zero transposes:
    xhl[g, p, c*128 + t]        = Xhi_shard[g*128 + t, c*128 + p]
    xhl[g, p, H + c*128 + t]    = Xlo_shard[g*128 + t, c*128 + p]
    whl[p, c*128 + e]           = Whi[e, c*128 + p]   (e < 64)
    whl[p, c*128 + 64 + e]      = Wlo[e, c*128 + p]
Each tile is one 2 MiB DMA (16 KiB contiguous per partition), alternating
between the two HWDGE rings (SP/ACT); outputs stream back incrementally on
the ACT ring, overlapped with the input stream.
The epilogue batch schedule shrinks toward the end (4,4,4,2,1,1) so the
serial chain after the last matmul is short, and psum_bufs=2 backpressures
epilogues to run early instead of stacking up in the tail.
Measured: ~115-125 us HW exec per core (best 115.1); the input stream
(32 MiB/core over ~358 GB/s HBM-per-core) runs at ~96% DMA busy, which is the
roofline for this shape. Startup preamble (~8 us) and the Tile drain/barrier
tail (~10 us) are framework-fixed costs.
"""

import numpy as np

import concourse.bass as bass
import concourse.bacc as bacc
import concourse.mybir as mybir
from concourse.tile import TileContext
from concourse.bass_utils import run_bass_kernel_spmd

T, H, E, TOPK = 16384, 4096, 64, 8
NCORES = 8
TC = T // NCORES          # 2048 tokens per core
PT = 128                  # tokens per tile (partition dim)
NG = TC // PT             # 16 tiles per core
NCH = H // 128            # 32 contraction chunks
BATCH_SCHEDULE = (4, 4, 4, 2, 1, 1)  # tiles per PSUM-bank batch; small tail
WSCALE = 64.0             # power-of-two W prescale (exactly undone on device)

F32 = mybir.dt.float32
F16 = mybir.dt.float16
U32 = mybir.dt.uint32


def build(x_bufs: int = 8, psum_bufs: int = 2, alt_rings: bool = True):
    nc = bacc.Bacc()
    xhl = nc.dram_tensor("xhl", [NG, 128, 2 * H], F16, kind="ExternalInput")
    whl = nc.dram_tensor("whl", [128, NCH * 128], F16, kind="ExternalInput")
    logits = nc.dram_tensor("logits", [TC, E], F32, kind="ExternalOutput")
    weights = nc.dram_tensor("weights", [TC, E], F32, kind="ExternalOutput")
    indices = nc.dram_tensor("indices", [TC, TOPK], U32, kind="ExternalOutput")

    inv = 1.0 / WSCALE
    lg_view = logits.rearrange("(g p) e -> p g e", p=128)
    wg_view = weights.rearrange("(g p) e -> p g e", p=128)
    ix_view = indices.rearrange("(g p) k -> p g k", p=128)

    with TileContext(nc) as tc:
        with (
            tc.tile_pool(name="xp", bufs=x_bufs) as xp,
            tc.tile_pool(name="wp", bufs=1) as wp,
            tc.tile_pool(name="pp", bufs=psum_bufs, space="PSUM") as pp,
            tc.tile_pool(name="res", bufs=1) as res,
            tc.tile_pool(name="tmp", bufs=2) as tmp,
            tc.tile_pool(name="st", bufs=3) as st,
        ):
            # weights on the ACT HWDGE ring so the SP ring starts streaming X
            # immediately
            wt_sb = wp.tile([128, NCH * 128], F16)
            nc.scalar.dma_start(out=wt_sb[:], in_=whl[:, :])

            lg_all = res.tile([128, NG * E], F32, tag="lg")
            wg_all = res.tile([128, NG * E], F32, tag="wg")
            idx_all = res.tile([128, NG * TOPK], U32, tag="idx")

            # full-width batches in the body; shrinking batches at the end so
            # the final epilogue chain (which serializes after the last
            # matmul) is as short as possible
            batches = []
            g0 = 0
            for tb_n in BATCH_SCHEDULE:
                batches.append((g0, tb_n))
                g0 += tb_n
            assert g0 == NG

            for g0, tb_n in batches:
                xgs = []
                for tb in range(tb_n):
                    g = g0 + tb
                    xg = xp.tile([128, 2 * H], F16, tag="xg")
                    # alternate the two HWDGE rings so descriptor-gen and
                    # completion handling pipeline across rings
                    eng = nc.sync if (g % 2 == 0 or not alt_rings) else nc.scalar
                    eng.dma_start(out=xg[:], in_=xhl[g])
                    xgs.append(xg)

                # one PSUM bank holds tb_n tiles x [Whi-cols | Wlo-cols]
                ps = pp.tile([128, tb_n * 2 * E], F32, tag="ps")
                for tb in range(tb_n):
                    xg = xgs[tb]
                    o = tb * 2 * E
                    for c in range(NCH):
                        hi = xg[:, c * 128:(c + 1) * 128]
                        lo = xg[:, H + c * 128:H + (c + 1) * 128]
                        whilo = wt_sb[:, c * 128:(c + 1) * 128]
                        whi = wt_sb[:, c * 128:c * 128 + 64]
                        nc.tensor.matmul(
                            ps[:, o:o + 2 * E], lhsT=hi, rhs=whilo,
                            start=(c == 0), stop=False, skip_group_check=True,
                        )
                        nc.tensor.matmul(
                            ps[:, o:o + E], lhsT=lo, rhs=whi,
                            start=False, stop=(c == NCH - 1),
                            skip_group_check=True,
                        )

                # ---- batched epilogue over tb_n tiles ----
                ps3 = ps[:].rearrange("p (t u) -> p t u", u=2 * E)
                half = tmp.tile([128, tb_n * E], F32, tag="half")
                half3 = half[:].rearrange("p (t u) -> p t u", u=E)
                nc.scalar.activation(
                    out=half3, in_=ps3[:, :, E:2 * E],
                    func=mybir.ActivationFunctionType.Copy,
                )
                lgp = tmp.tile([128, tb_n * E], F32, tag="lgp")
                lgp3 = lgp[:].rearrange("p (t u) -> p t u", u=E)
                nc.vector.tensor_add(lgp3, ps3[:, :, 0:E], half3)

                # exp first: the logits copy below is not needed by the
                # softmax/top-k chain, keep it off the critical path
                ex = tmp.tile([128, tb_n * E], F32, tag="ex")
                nc.scalar.activation(
                    out=ex[:], in_=lgp[:],
                    func=mybir.ActivationFunctionType.Exp, scale=inv,
                )
                ex3 = ex[:].rearrange("p (t u) -> p t u", u=E)

                s4 = st.tile([128, tb_n], F32, tag="s4")
                nc.vector.tensor_reduce(
                    out=s4[:], in_=ex3, axis=mybir.AxisListType.X,
                    op=mybir.AluOpType.add,
                )
                r4 = st.tile([128, tb_n], F32, tag="r4")
                nc.vector.reciprocal(r4[:], s4[:])

                wgf = wg_all[:, g0 * E:(g0 + tb_n) * E]
                wg3 = wgf.rearrange("p (t u) -> p t u", u=E)
                nc.vector.tensor_mul(wg3, ex3, r4[:].to_broadcast([128, tb_n, E]))

                for tb in range(tb_n):
                    g = g0 + tb
                    wg = wg_all[:, g * E:(g + 1) * E]
                    top8 = st.tile([128, TOPK], F32, tag="top8")
                    nc.vector.max(out=top8[:], in_=wg)
                    idx = idx_all[:, g * TOPK:(g + 1) * TOPK]
                    nc.vector.max_index(out=idx, in_max=top8[:], in_values=wg)

                lg = lg_all[:, g0 * E:(g0 + tb_n) * E]
                nc.scalar.activation(
                    out=lg, in_=lgp[:],
                    func=mybir.ActivationFunctionType.Copy, scale=inv,
                )

                # incremental writeback on the ACT ring, overlapped with the
                # input stream; indices first (end of the critical chain)
                gs = slice(g0, g0 + tb_n)
                fs = slice(g0 * E, (g0 + tb_n) * E)
                ks = slice(g0 * TOPK, (g0 + tb_n) * TOPK)
                nc.scalar.dma_start(out=ix_view[:, gs, :], in_=idx_all[:, ks])
                nc.scalar.dma_start(out=wg_view[:, gs, :], in_=wg_all[:, fs])
                nc.scalar.dma_start(out=lg_view[:, gs, :], in_=lg_all[:, fs])

    nc.finalize()
    return nc


_NC_CACHE = None
LAST_EXEC_NS = None


def _get_nc():
    global _NC_CACHE
    if _NC_CACHE is None:
        _NC_CACHE = build()
    return _NC_CACHE


def _prep_w(W: np.ndarray):
    # W prescale + fp16 hi/lo split, contraction-major: whl[p, c, 0:64|64:128]
    Wp = (W * WSCALE).astype(np.float32)
    W1 = np.ascontiguousarray(Wp.reshape(E, NCH, 128).transpose(2, 1, 0))  # [128,NCH,64]
    whi = W1.astype(np.float16)
    wlo = (W1 - whi.astype(np.float32)).astype(np.float16)
    return np.concatenate([whi, wlo], axis=2).reshape(128, NCH * 128)


def _prep_x_numpy(hidden_states: np.ndarray):
    xhi_full = hidden_states.astype(np.float16)
    xlo_full = (hidden_states - xhi_full.astype(np.float32)).astype(np.float16)
    out = []
    for core in range(NCORES):
        xhl = np.empty((NG, 128, 2 * H), np.float16)
        dst_hi = xhl[:, :, :H].reshape(NG, 128, NCH, PT)
        dst_lo = xhl[:, :, H:].reshape(NG, 128, NCH, PT)
        sl = slice(core * TC, (core + 1) * TC)
        # [g, t, c, p] -> [g, p, c, t]
        dst_hi[...] = xhi_full[sl].reshape(NG, PT, NCH, 128).transpose(0, 3, 2, 1)
        dst_lo[...] = xlo_full[sl].reshape(NG, PT, NCH, 128).transpose(0, 3, 2, 1)
        out.append(xhl)
    return out


def _prep_x(hidden_states: np.ndarray):
    # XLA-CPU does the big permute blocked + multithreaded (~4x numpy)
    try:
        import jax
        import jax.numpy as jnp

        cpu = jax.devices("cpu")[0]

        @jax.jit
        def prep_all(x):
            xt = x.reshape(NCORES, NG, PT, NCH, 128).transpose(0, 1, 4, 3, 2)
            xt = xt.reshape(NCORES, NG, 128, H)
            hi = xt.astype(jnp.float16)
            lo = (xt - hi.astype(jnp.float32)).astype(jnp.float16)
            return jnp.concatenate([hi, lo], axis=3)

        with jax.default_device(cpu):
            out = np.asarray(prep_all(hidden_states))
        return [out[core] for core in range(NCORES)]
    except Exception:
        return _prep_x_numpy(hidden_states)


def _prep_core_inputs(hidden_states: np.ndarray, W: np.ndarray):
    whl = _prep_w(W)
    xs = _prep_x(hidden_states)
    return [{"xhl": xs[core], "whl": whl} for core in range(NCORES)]


def kernel(hidden_states: np.ndarray, W: np.ndarray):
    hidden_states = np.ascontiguousarray(hidden_states, dtype=np.float32)
    W = np.ascontiguousarray(W, dtype=np.float32)
    assert hidden_states.shape == (T, H) and W.shape == (E, H)

    nc = _get_nc()
    in_maps = _prep_core_inputs(hidden_states, W)
    res = run_bass_kernel_spmd(nc, in_maps, core_ids=list(range(NCORES)))
    global LAST_EXEC_NS
    if res.exec_time_ns is not None:
        LAST_EXEC_NS = res.exec_time_ns

    logits = np.concatenate([res.results[i]["logits"] for i in range(NCORES)], axis=0)
    weights = np.concatenate([res.results[i]["weights"] for i in range(NCORES)], axis=0)
    indices = np.concatenate(
        [res.results[i]["indices"] for i in range(NCORES)], axis=0
    ).astype(np.int32)
    return logits, weights, indices
